# revision 1
# baseline (speedup 1.0000x reference)
"""MultiHeadLinearAttention (Linformer-style) on 8 trn2 NeuronCores.

Strategy (head-parallel attention + AllToAll + token-parallel output proj):
  - 16 heads -> 8 cores, 2 heads (one d_model slice of 128) per core.
  - Per core, per batch b:
      Kp  [128(d2), 256]   = K_slice^T @ We          (+be)
      Vp  [256(k), 128(d2)] = Wf^T @ V_slice         (+bf)
      s^T [256(k), n]       = Kp_h^T @ Q_h^T  (Q^T prepared on host)
      E^T = exp(s^T / 8)    (softmax without max-subtraction; |s| <= ~6)
      at  [65, n] = [Vp_h | 1]^T @ E^T   (row 64 = softmax denominator)
      attn^T[d2, n] = at[0:64] * (1/at[64]) broadcast
  - AllToAll exchanges attn^T n-chunks so core i ends with ALL d_model rows
    for tokens [512*i, 512*(i+1)).  Done in 2 halves (b in {0,1} / {2,3}) so
    comm overlaps compute.
  - Per core: out[n_shard] = attn_full^T.T @ Wo (+bo).

All matmuls run in bf16 (inputs host-cast) with fp32 PSUM accumulation.
"""

import numpy as np
import ml_dtypes

import concourse.bass as bass
import concourse.mybir as mybir
from concourse.tile import TileContext
from concourse.bass_utils import run_bass_kernel_spmd
from concourse.masks import make_identity
from concourse.tile_rust import add_dep_helper

B, N, D, H, LK = 4, 4096, 1024, 16, 256
DK = D // H          # 64
NC = 8               # cores
NSH = N // NC        # 512 tokens per core in phase 3
P = 128
NCH = 32             # 128-row chunks of N
NPH = 8              # 512-col chunks of N

F32 = mybir.dt.float32
F32R = mybir.dt.float32r
BF16 = mybir.dt.bfloat16
NP_BF16 = ml_dtypes.bfloat16

_BUILD_CACHE = {}

_ws_ctr = [0]


def _split_multi_waits(nc, lim=1):
    """Walrus codegen on this stack rejects instructions whose on_wait list
    exceeds the per-format wait-slot count ("Too many sync wait commands").
    Engines execute in order, so excess waits move onto preceding NOPs on
    the same engine with identical semantics."""
    for f in nc.m.functions:
        for blk in f.blocks:
            insts = blk.instructions
            if not any(
                ins.sync_info is not None and len(ins.sync_info.on_wait or []) > lim
                for ins in insts
            ):
                continue
            out = []
            for ins in insts:
                si = ins.sync_info
                waits = list(si.on_wait) if si is not None and si.on_wait else []
                if len(waits) > lim and ins.engine is not None:
                    keep = waits[-lim:]
                    rest = waits[:-lim]
                    while rest:
                        chunk, rest = rest[:lim], rest[lim:]
                        _ws_ctr[0] += 1
                        nop = mybir.InstNoOp(
                            name=f"I-waitsplit-{_ws_ctr[0]}", ins=[], outs=[]
                        )
                        nop.engine = ins.engine
                        nop.sync_info = mybir.SyncInfo(on_wait=chunk, on_update=[])
                        out.append(nop)
                    ins.sync_info = mybir.SyncInfo(
                        on_wait=keep, on_update=list(si.on_update or [])
                    )
                out.append(ins)
            blk.instructions = out
    return nc


def _build(use_be, use_bf, use_bo):
    nc = bass.Bass(num_devices=NC)

    Ks_p = nc.declare_dram_parameter("Ks", [B, N, P], BF16, isOutput=False)
    Vs_p = nc.declare_dram_parameter("Vs", [B, N, P], BF16, isOutput=False)
    QT_p = nc.declare_dram_parameter("QTs", [B, P, N], BF16, isOutput=False)
    We_p = nc.declare_dram_parameter("We", [N, LK], BF16, isOutput=False)
    Wf_p = nc.declare_dram_parameter("Wf", [N, LK], BF16, isOutput=False)
    Wo_p = nc.declare_dram_parameter("Wo", [D, D], BF16, isOutput=False)
    if use_be:
        be_p = nc.declare_dram_parameter("beB", [P, LK], F32, isOutput=False)
    if use_bf:
        bf_p = nc.declare_dram_parameter("bfB", [P, 2], F32, isOutput=False)
    if use_bo:
        bo_p = nc.declare_dram_parameter("boB", [P, D], F32, isOutput=False)
    out_p = nc.declare_dram_parameter("out", [B, NSH, D], F32, isOutput=True)

    rg = [list(range(NC))]

    with TileContext(nc) as tc:
        with (
            tc.tile_pool(name="wpool", bufs=1) as wpool,
            tc.tile_pool(name="state", bufs=1) as state,
            tc.tile_pool(name="dram", bufs=1, space="DRAM") as dram,
        ):
            # ---- resident weights (chunked so phase-1 matmuls start early)
            We_r = We_p.rearrange("(o p) k -> p o k", p=P)
            Wf_r = Wf_p.rearrange("(o p) k -> p o k", p=P)
            We_sb = wpool.tile([P, NCH, LK], BF16)
            Wf_sb = wpool.tile([P, NCH, LK], BF16)
            for ch in range(4):
                cs = slice(ch * (NCH // 4), (ch + 1) * (NCH // 4))
                nc.sync.dma_start(We_sb[:, cs, :], We_r[:, cs, :])
                nc.sync.dma_start(Wf_sb[:, cs, :], Wf_r[:, cs, :])
            Wo_sb = wpool.tile([P, D // P, D], BF16)
            if use_be:
                be_sb = wpool.tile([P, LK], F32)
                nc.sync.dma_start(be_sb[:], be_p[:])
            if use_bf:
                bf_sb = wpool.tile([P, 2], F32)
                nc.sync.dma_start(bf_sb[:], bf_p[:])

            # ---- A2A buffers (two halves for comm/compute overlap)
            a2a_in = [
                dram.tile([NC, P, 2, NSH], BF16, name=f"a2a_in{i}") for i in range(2)
            ]
            a2a_out = [
                dram.tile([NC, P, 2, NSH], BF16, name=f"a2a_out{i}") for i in range(2)
            ]

            # persistent per-core attention state
            # Kp_pad[p=d2(zero-padded per head), b, h, kc, 128(k)]
            Kp_pad = state.tile([P, B, 2, 2, P], BF16)
            # Vp_aug[p=k, kc, b, h, 65(d|1)]
            Vp_aug = state.tile([P, 2, B, 2, DK + 1], BF16)
            # identity for PE transposes (denominator batching)
            ident = state.tile([P, P], F32)
            make_identity(nc, ident[:])
            # selector for the denominator-broadcast matmul:
            # SEL[p, r, d] = 1 iff p == r; rb_r = SEL[:, r, :].T @ rden_all
            sel_f = state.tile([16, 16, DK], F32)
            nc.gpsimd.memset(sel_f[:], 0.0)
            nc.gpsimd.affine_select(
                out=sel_f[:],
                in_=sel_f[:],
                compare_op=mybir.AluOpType.not_equal,
                fill=1.0,
                base=0,
                # val = p - r (+0*d); fill 1.0 where p == r
                pattern=[[-1, 16], [0, DK]],
                channel_multiplier=1,
            )
            SEL = state.tile([16, 16, DK], F32R)
            nc.vector.tensor_copy(SEL[:], sel_f[:])

            # ================= phase 1: Kp / Vp =================
            with (
                tc.tile_pool(name="p1", bufs=3) as p1,
                tc.tile_pool(name="p1ps", bufs=1, space="PSUM") as p1ps,
            ):
                kp_ps = [
                    p1ps.tile([P, LK], F32, name=f"kp{b}", tag=f"kp{b}")
                    for b in range(B)
                ]
                vp_ps = [
                    p1ps.tile([P, B * P], F32, name=f"vp{kc}", tag=f"vp{kc}")
                    for kc in range(2)
                ]
                for ic in range(NCH):
                    K4 = p1.tile([P, B, P], BF16, name="K4", tag="K4")
                    nc.sync.dma_start(
                        K4[:],
                        Ks_p[:, ic * P : (ic + 1) * P, :].rearrange("b n d -> n b d"),
                    )
                    V4 = p1.tile([P, B, P], BF16, name="V4", tag="V4")
                    nc.sync.dma_start(
                        V4[:],
                        Vs_p[:, ic * P : (ic + 1) * P, :].rearrange("b n d -> n b d"),
                    )
                    for b in range(B):
                        nc.tensor.matmul(
                            kp_ps[b][:],
                            K4[:, b, :],
                            We_sb[:, ic, :],
                            start=(ic == 0),
                            stop=(ic == NCH - 1),
                        )
                    for kc in range(2):
                        nc.tensor.matmul(
                            vp_ps[kc][:],
                            Wf_sb[:, ic, kc * P : (kc + 1) * P],
                            V4[:],
                            start=(ic == 0),
                            stop=(ic == NCH - 1),
                        )

                # epilogue: build Kp_pad (zero-padded per head) and Vp_aug
                nc.vector.memset(Kp_pad[:], 0.0)
                nc.vector.memset(Vp_aug[:, :, :, :, DK : DK + 1], 1.0)
                for b in range(B):
                    for h in range(2):
                        hs = slice(h * DK, (h + 1) * DK)
                        for kc in range(2):
                            ks = slice(kc * P, (kc + 1) * P)
                            if use_be:
                                nc.vector.tensor_tensor(
                                    Kp_pad[hs, b, h, kc, :],
                                    kp_ps[b][hs, ks],
                                    be_sb[hs, ks],
                                    mybir.AluOpType.add,
                                )
                            else:
                                nc.vector.tensor_copy(
                                    Kp_pad[hs, b, h, kc, :], kp_ps[b][hs, ks]
                                )
                for kc in range(2):
                    for b in range(B):
                        for h in range(2):
                            src = vp_ps[kc][:, b * P + h * DK : b * P + (h + 1) * DK]
                            dst = Vp_aug[:, kc, b, h, 0:DK]
                            if use_bf:
                                nc.vector.tensor_scalar_add(
                                    dst, src, bf_sb[:, kc : kc + 1]
                                )
                            else:
                                nc.vector.tensor_copy(dst, src)

            # ================= phase 2: scores/softmax/attn, + A2A =================
            cc_insts = []
            for half in range(2):
                with (
                    tc.tile_pool(name=f"p2_{half}", bufs=3) as p2,
                    tc.tile_pool(name=f"p2ps_{half}", bufs=1, space="PSUM") as p2ps,
                ):
                    for b2 in range(2):
                        b = half * 2 + b2
                        # [16, NSH] f32: row r = nh*2+h holds that instance's
                        # softmax denominators (collected by DMA from PSUM)
                        den_all = p2.tile(
                            [16, NSH], F32, name="den_all", tag="den_all", bufs=2
                        )
                        araw = {}
                        for nh in range(NPH):
                            QT2 = p2.tile([P, NSH], BF16, name="QT2", tag="QT2", bufs=3)
                            nc.sync.dma_start(
                                QT2[:], QT_p[b, :, nh * NSH : (nh + 1) * NSH]
                            )
                            ET = [
                                p2.tile(
                                    [P, 2, NSH], BF16, name=f"ET{h}", tag=f"ET{h}",
                                    bufs=2,
                                )
                                for h in range(2)
                            ]
                            for h in range(2):
                                for kc in range(2):
                                    st = p2ps.tile(
                                        [P, NSH], F32, name="st", tag="st", bufs=2
                                    )
                                    nc.tensor.matmul(
                                        st[:],
                                        Kp_pad[:, b, h, kc, :],
                                        QT2[:],
                                        start=True,
                                        stop=True,
                                    )
                                    nc.scalar.activation(
                                        ET[h][:, kc, :],
                                        st[:],
                                        mybir.ActivationFunctionType.Exp,
                                        scale=0.125,
                                    )
                            for h in range(2):
                                at = p2ps.tile(
                                    [DK + 1, NSH], F32, name="at", tag="at", bufs=2
                                )
                                for kc in range(2):
                                    nc.tensor.matmul(
                                        at[:],
                                        Vp_aug[:, kc, b, h, :],
                                        ET[h][:, kc, :],
                                        start=(kc == 0),
                                        stop=(kc == 1),
                                    )
                                r = nh * 2 + h
                                den_r = p2.tile(
                                    [1, NSH], F32, name="den_r", tag="den_r", bufs=3
                                )
                                nc.scalar.copy(den_r[:], at[DK : DK + 1, :])
                                nc.sync.dma_start(den_all[r : r + 1, :], den_r[:])
                                ar = p2.tile(
                                    [DK, NSH], F32, name="araw", tag="araw", bufs=18
                                )
                                nc.vector.tensor_copy(ar[:], at[0:DK, :])
                                araw[r] = ar
                        # batched reciprocal: transpose dens to [128, 64],
                        # one 64-free-element DVE reciprocal, transpose back.
                        denT = p2ps.tile([P, 64], F32, name="denT", tag="denT",
                                         bufs=1)
                        for blk in range(4):
                            nc.tensor.transpose(
                                denT[:, blk * 16 : (blk + 1) * 16],
                                den_all[:, blk * P : (blk + 1) * P],
                                ident[0:16, 0:16],
                            )
                        rT = p2.tile([P, 64], F32, name="rT", tag="rT")
                        nc.vector.reciprocal(rT[:], denT[:])
                        rden_ps = p2ps.tile([16, NSH], F32, name="rden_ps",
                                            tag="rden_ps", bufs=1)
                        for blk in range(4):
                            nc.tensor.transpose(
                                rden_ps[:, blk * P : (blk + 1) * P],
                                rT[:, blk * 16 : (blk + 1) * 16],
                                ident[:],
                            )
                        rden_sb = p2.tile([16, NSH], F32R, name="rden_sb",
                                          tag="rden_sb", bufs=2)
                        nc.vector.tensor_copy(rden_sb[:], rden_ps[:])
                        for nh in range(NPH):
                            for h in range(2):
                                r = nh * 2 + h
                                rb = p2ps.tile([DK, NSH], F32, name="rb", tag="rb",
                                               bufs=2)
                                nc.tensor.matmul(
                                    rb[:],
                                    SEL[:, r, :],
                                    rden_sb[:],
                                    start=True,
                                    stop=True,
                                )
                                attn_sb = p2.tile(
                                    [DK, NSH], BF16, name="attn_sb", tag="attn_sb"
                                )
                                nc.vector.tensor_tensor(
                                    attn_sb[:],
                                    araw[r][:],
                                    rb[:],
                                    mybir.AluOpType.mult,
                                )
                                nc.sync.dma_start(
                                    a2a_in[half][nh, h * DK : (h + 1) * DK, b2, :],
                                    attn_sb[:],
                                )
                cc = nc.gpsimd.collective_compute(
                    "AllToAll",
                    mybir.AluOpType.bypass,
                    replica_groups=rg,
                    ins=[a2a_in[half][:]],
                    outs=[a2a_out[half][:]],
                )
                cc_insts.append(cc.ins)

            # ================= phase 3: output projection =================
            with (
                tc.tile_pool(name="p3", bufs=3) as p3,
                tc.tile_pool(name="p3ps", bufs=1, space="PSUM") as p3ps,
            ):
                nc.sync.dma_start(
                    Wo_sb[:], Wo_p.rearrange("(o p) j -> p o j", p=P)
                )
                if use_bo:
                    bo_sb = wpool.tile([P, D], F32)
                    nc.sync.dma_start(bo_sb[:], bo_p[:])
                for half in range(2):
                    for b2 in range(2):
                        b = half * 2 + b2
                        gs = []
                        for dm in range(D // P):
                            g = p3.tile([P, NSH], BF16, name="g", tag="g", bufs=10)
                            gdma = nc.sync.dma_start(
                                g[:], a2a_out[half][dm, :, b2, :]
                            )
                            if half == 0:
                                # keep these DMAs' queue slots behind the 2nd
                                # collective's inputs: otherwise they head-of-
                                # line-block the a2a_in[1] writes the second
                                # AllToAll trigger waits on.
                                add_dep_helper(
                                    gdma.ins,
                                    cc_insts[1],
                                    sync=False,
                                    reason="order g-loads after 2nd collective",
                                )
                            gs.append(g)
                        for mt in range(NSH // P):
                            f = [
                                p3ps.tile(
                                    [P, 512], F32, name=f"f{fi}", tag="f", bufs=4
                                )
                                for fi in range(2)
                            ]
                            for dm in range(D // P):
                                for fi in range(2):
                                    nc.tensor.matmul(
                                        f[fi][:],
                                        gs[dm][:, mt * P : (mt + 1) * P],
                                        Wo_sb[:, dm, fi * 512 : (fi + 1) * 512],
                                        start=(dm == 0),
                                        stop=(dm == D // P - 1),
                                    )
                            osb = p3.tile([P, D], F32, name="osb", tag="osb")
                            for fi in range(2):
                                if use_bo:
                                    nc.vector.tensor_tensor(
                                        osb[:, fi * 512 : (fi + 1) * 512],
                                        f[fi][:],
                                        bo_sb[:, fi * 512 : (fi + 1) * 512],
                                        mybir.AluOpType.add,
                                    )
                                else:
                                    nc.vector.tensor_copy(
                                        osb[:, fi * 512 : (fi + 1) * 512], f[fi][:]
                                    )
                            nc.sync.dma_start(
                                out_p[b, mt * P : (mt + 1) * P, :], osb[:]
                            )

    return nc


def kernel(K, Q, V, We, be, Wf, bf, Wo, bo, n_heads, _trace=False):
    assert int(n_heads) == H
    K = np.asarray(K, np.float32)
    Q = np.asarray(Q, np.float32)
    V = np.asarray(V, np.float32)
    We = np.asarray(We, np.float32)
    be = np.asarray(be, np.float32)
    Wf = np.asarray(Wf, np.float32)
    bf = np.asarray(bf, np.float32)
    Wo = np.asarray(Wo, np.float32)
    bo = np.asarray(bo, np.float32)

    use_be = bool(np.any(be))
    use_bf = bool(np.any(bf))
    use_bo = bool(np.any(bo))

    key = (use_be, use_bf, use_bo)
    if key not in _BUILD_CACHE:
        _BUILD_CACHE[key] = _split_multi_waits(_build(*key))
    nc = _BUILD_CACHE[key]

    Kb = K.astype(NP_BF16)
    Vb = V.astype(NP_BF16)
    Qb = Q.astype(NP_BF16)
    Web = We.astype(NP_BF16)
    Wfb = Wf.astype(NP_BF16)
    Wob = Wo.astype(NP_BF16)

    in_maps = []
    for c in range(NC):
        cs = slice(P * c, P * (c + 1))
        m = {
            "Ks": np.ascontiguousarray(Kb[:, :, cs]),
            "Vs": np.ascontiguousarray(Vb[:, :, cs]),
            "QTs": np.ascontiguousarray(Qb[:, :, cs].transpose(0, 2, 1)),
            "We": Web,
            "Wf": Wfb,
            "Wo": Wob,
        }
        if use_be:
            m["beB"] = np.broadcast_to(be, (P, LK)).copy()
        if use_bf:
            m["bfB"] = np.ascontiguousarray(bf.reshape(2, P).T)
        if use_bo:
            m["boB"] = np.broadcast_to(bo, (P, D)).copy()
        in_maps.append(m)

    res = run_bass_kernel_spmd(nc, in_maps, list(range(NC)), trace=_trace)

    out = np.empty((B, N, D), np.float32)
    for c in range(NC):
        out[:, NSH * c : NSH * (c + 1), :] = res.results[c]["out"]
    if _trace:
        kernel._last_exec_time_ns = res.exec_time_ns
    return out


kernel._last_exec_time_ns = None

